# revision 13
# baseline (speedup 1.0000x reference)
"""LoRA Linear (residual + low-rank path with dropout) on 8 Trainium2 cores.

Math (fp32 reference):
  residual = hidden_states @ W_base.T
  dropped  = hidden_states * dropout_mask / (1 - p)
  out      = residual + ((dropped @ A.T) @ B.T) * scaling

Sharding: data-parallel over the 8192 tokens (8 cores x 1024 tokens);
W_base / A / B replicated.  All matmuls run on the PE in float32r.

v5 layout: stationary = W o-tile [128k x 128o], moving = 512-token
slice of x [128k x 512t], PSUM out = [128o x 512t].  Each PSUM bank
receives a k-contiguous run of matmuls (alternating banks per matmul
breaks the PE's LDWEIGHTS chase: 272 vs 227 ns/MM measured).  The LoRA
accumulate is zero-padded to K=128 so every PE instruction is a
homogeneous [128x128]x[128x512] matmul (a K=16 group tail measured
~+400 ns per group).

v5 phasing fixes the DMA-bound startup of v4 (first MM at 24 us plus
42 us of piece-wait gaps): x streams token-half th0 first, and the
prologue runs the first three o-tiles on th0 only -- the same PE work
per streamed x byte from half-size pieces (SBUF caps the f32r W pool
at 3 tiles, which also caps the early o-tile count).  While th1 streams, phase B runs th0-only o-tile sweeps
(injecting LoRA stage-1 th1 chunks as pieces arrive); phase C runs
both halves per o-tile; phase D finishes the th1 halves of the early
o-tiles with re-fetched W (total DMA stays far under the compute
roofline).

Queues: x/mask on the SP HWDGE ring; W0 on the ACT HWDGE ring; the W
stream and per-o-tile B tiles on the Pool/GpSimd SWDGE ring; output
DMAs on ACT; DVE does mask-multiplies and PSUM drains.
"""

import numpy as np

P = 128
D_IN = 4096
D_OUT = 4096
BATCH, SEQ = 4, 2048
TOK = BATCH * SEQ  # 8192
NCORES = 8
T = TOK // NCORES  # 1024 tokens per core, all resident
KT = D_IN // P  # 32 k-tiles
OT = D_OUT // P  # 32 out-tiles of 128
TH = 2  # token halves (512 each)
NF = T // TH  # 512 moving free dim
KP = 8  # x/mask DMA pieces per half (4 k-tiles each)
KPK = KT // KP
R = 16
DROP_P = 0.05
SCALING = 32.0 / 16.0
E5 = 3  # o-tiles run on th0 during the prologue (= wt pool depth)
RB = 8  # first phase-C o-tile; ots E5..RB-1 are phase-B (th0-only here)
W_BUFS = 3
BT_BUFS = 4

_PROGRAM_CACHE = {}


def _build_program():
    from concourse import bacc
    import concourse.mybir as mybir
    import concourse.tile as tile

    f32 = mybir.dt.float32
    f32r = mybir.dt.float32r
    u8 = mybir.dt.uint8

    nc = bacc.Bacc("TRN2", target_bir_lowering=False)
    xT_d = nc.dram_tensor("xT", [TH, P, KT, NF], f32r, kind="ExternalInput")
    mT_d = nc.dram_tensor("mT", [TH, P, KT, NF], u8, kind="ExternalInput")
    WT_d = nc.dram_tensor("WT", [OT, P, KT, P], f32r, kind="ExternalInput")
    AT_d = nc.dram_tensor("AT", [P, KT, R], f32r, kind="ExternalInput")
    BT_d = nc.dram_tensor("BT", [OT, P, P], f32r, kind="ExternalInput")
    out_d = nc.dram_tensor("out", [OT, TH, P, NF], f32, kind="ExternalOutput")

    # bookkeeping to assert the emission covers everything exactly once
    mm_count = [0]
    k_done = {(ot, th): 0 for ot in range(OT) for th in range(TH)}

    with tile.TileContext(nc) as tc:
        with (
            tc.tile_pool(name="xt", bufs=1) as xtpool,
            tc.tile_pool(name="at", bufs=1) as atpool,
            tc.tile_pool(name="bt", bufs=BT_BUFS) as btpool,
            tc.tile_pool(name="wt", bufs=W_BUFS) as wtpool,
            tc.tile_pool(name="m", bufs=2) as mpool,
            tc.tile_pool(name="d", bufs=2) as dpool,
            tc.tile_pool(name="xa", bufs=1) as xapool,
            tc.tile_pool(name="z", bufs=1) as zpool,
            tc.tile_pool(name="o", bufs=2) as opool,
            tc.tile_pool(name="ps_xa", bufs=2, space="PSUM") as ps_xa,
            tc.tile_pool(name="ps_mm", bufs=6, space="PSUM") as ps_mm,
        ):
            xT_t = xtpool.tile([P, KT, T], f32r, tag="xT")
            AT_t = atpool.tile([P, KT, R], f32r, tag="AT")
            nc.sync.dma_start(AT_t[:], AT_d[:])

            WT_ts = {}

            def load_w(ot, queue, gen=""):
                WT_t = wtpool.tile([P, KT, P], f32r, tag="WT", name=f"WT{ot}{gen}")
                queue.dma_start(WT_t[:], WT_d[ot])
                WT_ts[ot] = WT_t

            BT_ts = {}

            def load_b(ot, gen=""):
                BT_t = btpool.tile([P, P], f32r, tag="BT", name=f"BT{ot}{gen}")
                nc.gpsimd.dma_start(BT_t[:], BT_d[ot])
                BT_ts[ot] = BT_t

            # ordered load queues: prologue W0..W{E5-1}, then the rest,
            # then re-fetches ("b" generation) for phase D
            w_order = [(i, "") for i in range(OT)] + [(i, "b") for i in range(RB)]
            b_order = list(w_order)
            w_next = [E5]
            b_next = [BT_BUFS]

            def pump_w():
                if w_next[0] < len(w_order):
                    i, gen = w_order[w_next[0]]
                    load_w2(i, gen)
                    w_next[0] += 1

            def pump_b():
                if b_next[0] < len(b_order):
                    i, gen = b_order[b_next[0]]
                    load_b(i, gen)
                    b_next[0] += 1

            def load_w2(ot, gen):
                load_w(ot, nc.gpsimd, gen)

            load_w(0, nc.scalar)
            for ot in range(1, E5):
                load_w(ot, nc.gpsimd)
            for ot in range(BT_BUFS):
                load_b(ot)

            # zero-padded xa (rows 16..127 stay 0); f32r inputs of a
            # matmul must be written AS f32r, so memset a f32 scratch
            # and round-copy it over
            xaT_t = xapool.tile([P, T], f32r, tag="xaT")
            z_t = zpool.tile([P, T], f32, tag="z")
            nc.vector.memset(z_t[:], 0.0)
            nc.vector.tensor_copy(xaT_t[:], z_t[:])

            xa_ps = [
                ps_xa.tile([R, NF], f32, tag="xa", name=f"xa_ps{h}")
                for h in range(TH)
            ]
            ps_early = {
                ot: ps_mm.tile([P, NF], f32, tag="ps", name=f"pse{ot}")
                for ot in range(E5)
            }

            def mm(ps, ot, th, k, start=False):
                ts = slice(th * NF, (th + 1) * NF)
                nc.tensor.matmul(
                    ps[:], WT_ts[ot][:, k], xT_t[:, k, ts],
                    start=start, stop=False,
                )
                mm_count[0] += 1
                assert k_done[(ot, th)] == k, (ot, th, k)
                k_done[(ot, th)] = k + 1

            m_ts = {}

            def dma_piece(th, kp):
                ks = slice(kp * KPK, (kp + 1) * KPK)
                tsl = slice(th * NF, (th + 1) * NF)
                nc.sync.dma_start(xT_t[:, ks, tsl], xT_d[th, :, ks])
                m_t = mpool.tile([P, KPK, NF], u8, tag="m", name=f"m{th}_{kp}")
                nc.sync.dma_start(m_t[:], mT_d[th, :, ks])
                m_ts[(th, kp)] = m_t

            def stage1_chunk(th, kp):
                ts = slice(th * NF, (th + 1) * NF)
                for k in range(kp * KPK, (kp + 1) * KPK):
                    d_t = dpool.tile([P, NF], f32r, tag="d", name=f"d{th}_{k}")
                    nc.vector.tensor_tensor(
                        d_t[:], xT_t[:, k, ts].bitcast(f32),
                        m_ts[(th, kp)][:, k - kp * KPK], mybir.AluOpType.mult,
                    )
                    nc.tensor.matmul(
                        xa_ps[th][:], AT_t[:, k], d_t[:],
                        start=(k == 0), stop=(k == KT - 1),
                    )
                    mm_count[0] += 1

            def xa_copy(th):
                nc.vector.tensor_copy(
                    xaT_t[:R, th * NF : (th + 1) * NF], xa_ps[th][:]
                )

            def close_group(ps, ot, th):
                ts = slice(th * NF, (th + 1) * NF)
                nc.tensor.matmul(
                    ps[:], BT_ts[ot][:], xaT_t[:, ts],
                    start=False, stop=True,
                )
                mm_count[0] += 1
                o_t = opool.tile([P, NF], f32, tag="o", name=f"o{ot}_{th}")
                nc.vector.tensor_copy(o_t[:], ps[:])
                nc.scalar.dma_start(out_d[ot, th], o_t[:])

            # ---- prologue: th0 x/mask pieces; staircase the five early
            # o-tiles' th0 k-runs so no matmul is issued before its W
            # tile can have arrived (W_i lands ~6us apart)
            backlog = {ot: 0 for ot in range(E5)}
            for kp in range(KP):
                dma_piece(0, kp)
                for ot in range(E5):
                    if ot <= kp + 1:
                        for k in range(backlog[ot], (kp + 1) * KPK):
                            mm(ps_early[ot], ot, 0, k, start=(k == 0))
                        backlog[ot] = (kp + 1) * KPK
                stage1_chunk(0, kp)

            # th1 x/mask stream (queued now, lands during phase B)
            for kp in range(KP):
                dma_piece(1, kp)

            xa_copy(0)
            # close the early o-tiles' th0 groups (frees their banks)
            for ot in range(E5):
                close_group(ps_early[ot], ot, 0)

            # ---- phase B: th0-only sweeps while th1 streams in;
            # inject LoRA stage-1 th1 chunks as their pieces arrive
            pump_w()
            pump_w()
            chunks = list(range(KP))
            nb = RB - E5
            for i, ot in enumerate(range(E5, RB)):
                pump_b()
                pump_w()
                ps = ps_mm.tile([P, NF], f32, tag="ps", name=f"ps{ot}_0")
                for k in range(KT):
                    mm(ps, ot, 0, k, start=(k == 0))
                # spread the 8 th1 chunks across the B o-tiles
                take = (KP * (i + 1)) // nb - (KP * i) // nb
                for _ in range(take):
                    stage1_chunk(1, chunks.pop(0))
                close_group(ps, ot, 0)
            assert not chunks
            xa_copy(1)

            # ---- phase C: both halves per o-tile
            for ot in range(RB, OT):
                pump_b()
                pump_w()
                for th in range(TH):
                    ps = ps_mm.tile([P, NF], f32, tag="ps", name=f"ps{ot}_{th}")
                    for k in range(KT):
                        mm(ps, ot, th, k, start=(k == 0))
                    close_group(ps, ot, th)

            # ---- phase D: th1 halves of the early o-tiles (W re-fetched)
            for ot in range(RB):
                pump_b()
                pump_w()
                ps = ps_mm.tile([P, NF], f32, tag="ps", name=f"ps{ot}_1b")
                for k in range(KT):
                    mm(ps, ot, 1, k, start=(k == 0))
                close_group(ps, ot, 1)

    for (ot, th), v in k_done.items():
        assert v == KT, (ot, th, v)
    assert mm_count[0] == OT * TH * (KT + 1) + TH * KT, mm_count[0]
    nc.finalize()
    return nc


def _get_program():
    if "nc" not in _PROGRAM_CACHE:
        _PROGRAM_CACHE["nc"] = _build_program()
    return _PROGRAM_CACHE["nc"]


def kernel(hidden_states, W_base, A, B, dropout_mask):
    from concourse.bass_utils import run_bass_kernel_spmd

    hs = np.ascontiguousarray(np.asarray(hidden_states, dtype=np.float32)).reshape(
        TOK, D_IN
    )
    mask = np.asarray(dropout_mask).reshape(TOK, D_IN)
    W = np.asarray(W_base, dtype=np.float32)
    A_ = np.asarray(A, dtype=np.float32)
    B_ = np.asarray(B, dtype=np.float32)

    #   WT[ot, p, k, o] = W[ot*P+o, k*P+p]
    WT = np.ascontiguousarray(
        W.reshape(OT, P, KT, P).transpose(0, 3, 2, 1)
    ).astype(np.float32)
    #   AT[p, k, r] = A[r, k*P+p] / (1-p)
    AT = np.ascontiguousarray(
        A_.T.reshape(KT, P, R).transpose(1, 0, 2) * np.float32(1.0 / (1.0 - DROP_P))
    ).astype(np.float32)
    #   BT[ot, r, o] = B[ot*P+o, r] * scaling, zero-padded to r=128
    BT = np.zeros((OT, P, P), dtype=np.float32)
    BT[:, :R, :] = B_.T.reshape(R, OT, P).transpose(1, 0, 2) * np.float32(SCALING)

    in_maps = []
    for c in range(NCORES):
        sl = slice(c * T, (c + 1) * T)
        #   xT[th, p, k, t] = x[c*T + th*NF + t, k*P+p]
        xc = hs[sl].T.reshape(KT, P, TH, NF).transpose(2, 1, 0, 3)
        mc = mask[sl].T.reshape(KT, P, TH, NF).transpose(2, 1, 0, 3)
        xT = np.ascontiguousarray(xc)
        mT = np.ascontiguousarray(mc).astype(np.uint8)
        in_maps.append({"xT": xT, "mT": mT, "WT": WT, "AT": AT, "BT": BT})

    nc = _get_program()
    res = run_bass_kernel_spmd(nc, in_maps, core_ids=list(range(NCORES)))
    _PROGRAM_CACHE["last_results"] = res

    # out_dev[ot, th, o, t] = out[th*NF+t, ot*P+o]  (per core)
    parts = []
    for c in range(NCORES):
        od = res.results[c]["out"]  # [OT, TH, P, NF]
        parts.append(od.transpose(1, 3, 0, 2).reshape(T, D_OUT))
    out = np.concatenate(parts, axis=0)
    return out.reshape(BATCH, SEQ, D_OUT).astype(np.float32)


# revision 14
# speedup vs baseline: 1.0090x; 1.0090x over previous
"""LoRA Linear (residual + low-rank path with dropout) on 8 Trainium2 cores.

Math (fp32 reference):
  residual = hidden_states @ W_base.T
  dropped  = hidden_states * dropout_mask / (1 - p)
  out      = residual + ((dropped @ A.T) @ B.T) * scaling

Sharding: data-parallel over the 8192 tokens (8 cores x 1024 tokens);
W_base / A / B replicated.  All matmuls run on the PE in float32r.

v5 layout: stationary = W o-tile [128k x 128o], moving = 512-token
slice of x [128k x 512t], PSUM out = [128o x 512t].  Each PSUM bank
receives a k-contiguous run of matmuls (alternating banks per matmul
breaks the PE's LDWEIGHTS chase: 272 vs 227 ns/MM measured).  The LoRA
accumulate is zero-padded to K=128 so every PE instruction is a
homogeneous [128x128]x[128x512] matmul (a K=16 group tail measured
~+400 ns per group).

v5 phasing fixes the DMA-bound startup of v4 (first MM at 24 us plus
42 us of piece-wait gaps): x streams token-half th0 first, and the
prologue runs the first three o-tiles on th0 only -- the same PE work
per streamed x byte from half-size pieces (SBUF caps the f32r W pool
at 3 tiles, which also caps the early o-tile count).  While th1 streams, phase B runs th0-only o-tile sweeps
(injecting LoRA stage-1 th1 chunks as pieces arrive); phase C runs
both halves per o-tile; phase D finishes the th1 halves of the early
o-tiles with re-fetched W (total DMA stays far under the compute
roofline).

Queues: x/mask on the SP HWDGE ring; W0 on the ACT HWDGE ring; the W
stream and per-o-tile B tiles on the Pool/GpSimd SWDGE ring; output
DMAs on ACT; DVE does mask-multiplies and PSUM drains.
"""

import numpy as np

P = 128
D_IN = 4096
D_OUT = 4096
BATCH, SEQ = 4, 2048
TOK = BATCH * SEQ  # 8192
NCORES = 8
T = TOK // NCORES  # 1024 tokens per core, all resident
KT = D_IN // P  # 32 k-tiles
OT = D_OUT // P  # 32 out-tiles of 128
TH = 2  # token halves (512 each)
NF = T // TH  # 512 moving free dim
KP = 8  # x/mask DMA pieces per half (4 k-tiles each)
KPK = KT // KP
R = 16
DROP_P = 0.05
SCALING = 32.0 / 16.0
E5 = 3  # o-tiles run on th0 during the prologue (= wt pool depth)
RB = 8  # first phase-C o-tile; ots E5..RB-1 are phase-B (th0-only here)
W_BUFS = 3
BT_BUFS = 4

_PROGRAM_CACHE = {}


def _build_program():
    from concourse import bacc
    import concourse.mybir as mybir
    import concourse.tile as tile

    f32 = mybir.dt.float32
    f32r = mybir.dt.float32r
    u8 = mybir.dt.uint8

    nc = bacc.Bacc("TRN2", target_bir_lowering=False)
    xT_d = nc.dram_tensor("xT", [TH, P, KT, NF], f32r, kind="ExternalInput")
    mT_d = nc.dram_tensor("mT", [TH, P, KT, NF], u8, kind="ExternalInput")
    WT_d = nc.dram_tensor("WT", [OT, P, KT, P], f32r, kind="ExternalInput")
    AT_d = nc.dram_tensor("AT", [P, KT, R], f32r, kind="ExternalInput")
    BT_d = nc.dram_tensor("BT", [OT, P, P], f32r, kind="ExternalInput")
    out_d = nc.dram_tensor("out", [OT, TH, P, NF], f32, kind="ExternalOutput")

    # bookkeeping to assert the emission covers everything exactly once
    mm_count = [0]
    k_done = {(ot, th): 0 for ot in range(OT) for th in range(TH)}

    with tile.TileContext(nc) as tc:
        with (
            tc.tile_pool(name="xt", bufs=1) as xtpool,
            tc.tile_pool(name="at", bufs=1) as atpool,
            tc.tile_pool(name="bt", bufs=BT_BUFS) as btpool,
            tc.tile_pool(name="wt", bufs=W_BUFS) as wtpool,
            tc.tile_pool(name="m", bufs=2) as mpool,
            tc.tile_pool(name="d", bufs=2) as dpool,
            tc.tile_pool(name="xa", bufs=1) as xapool,
            tc.tile_pool(name="z", bufs=1) as zpool,
            tc.tile_pool(name="o", bufs=2) as opool,
            tc.tile_pool(name="ps_xa", bufs=2, space="PSUM") as ps_xa,
            tc.tile_pool(name="ps_mm", bufs=6, space="PSUM") as ps_mm,
        ):
            xT_t = xtpool.tile([P, TH, KT, NF], f32r, tag="xT")
            AT_t = atpool.tile([P, KT, R], f32r, tag="AT")
            nc.sync.dma_start(AT_t[:], AT_d[:])

            WT_ts = {}

            def load_w(ot, queue, gen=""):
                WT_t = wtpool.tile([P, KT, P], f32r, tag="WT", name=f"WT{ot}{gen}")
                queue.dma_start(WT_t[:], WT_d[ot])
                WT_ts[ot] = WT_t

            BT_ts = {}

            def load_b(ot, gen=""):
                BT_t = btpool.tile([P, P], f32r, tag="BT", name=f"BT{ot}{gen}")
                nc.gpsimd.dma_start(BT_t[:], BT_d[ot])
                BT_ts[ot] = BT_t

            # ordered load queues: prologue W0..W{E5-1}, then the rest,
            # then re-fetches ("b" generation) for phase D
            w_order = [(i, "") for i in range(OT)] + [(i, "b") for i in range(RB)]
            b_order = list(w_order)
            w_next = [E5]
            b_next = [BT_BUFS]

            def pump_w():
                if w_next[0] < len(w_order):
                    i, gen = w_order[w_next[0]]
                    load_w2(i, gen)
                    w_next[0] += 1

            def pump_b():
                if b_next[0] < len(b_order):
                    i, gen = b_order[b_next[0]]
                    load_b(i, gen)
                    b_next[0] += 1

            def load_w2(ot, gen):
                load_w(ot, nc.gpsimd, gen)

            load_w(0, nc.scalar)
            for ot in range(1, E5):
                load_w(ot, nc.gpsimd)
            for ot in range(BT_BUFS):
                load_b(ot)

            # zero-padded xa (rows 16..127 stay 0); f32r inputs of a
            # matmul must be written AS f32r, so memset a f32 scratch
            # and round-copy it over
            xaT_t = xapool.tile([P, T], f32r, tag="xaT")
            z_t = zpool.tile([P, T], f32, tag="z")
            nc.vector.memset(z_t[:], 0.0)
            nc.vector.tensor_copy(xaT_t[:], z_t[:])

            xa_ps = [
                ps_xa.tile([R, NF], f32, tag="xa", name=f"xa_ps{h}")
                for h in range(TH)
            ]
            ps_early = {
                ot: ps_mm.tile([P, NF], f32, tag="ps", name=f"pse{ot}")
                for ot in range(E5)
            }

            def mm(ps, ot, th, k, start=False):
                nc.tensor.matmul(
                    ps[:], WT_ts[ot][:, k], xT_t[:, th, k],
                    start=start, stop=False,
                )
                mm_count[0] += 1
                assert k_done[(ot, th)] == k, (ot, th, k)
                k_done[(ot, th)] = k + 1

            m_ts = {}

            def dma_piece(th, kp):
                ks = slice(kp * KPK, (kp + 1) * KPK)
                nc.sync.dma_start(xT_t[:, th, ks], xT_d[th, :, ks])
                m_t = mpool.tile([P, KPK, NF], u8, tag="m", name=f"m{th}_{kp}")
                nc.sync.dma_start(m_t[:], mT_d[th, :, ks])
                m_ts[(th, kp)] = m_t

            def stage1_chunk(th, kp):
                ts = slice(th * NF, (th + 1) * NF)
                for k in range(kp * KPK, (kp + 1) * KPK):
                    d_t = dpool.tile([P, NF], f32r, tag="d", name=f"d{th}_{k}")
                    nc.vector.tensor_tensor(
                        d_t[:], xT_t[:, th, k].bitcast(f32),
                        m_ts[(th, kp)][:, k - kp * KPK], mybir.AluOpType.mult,
                    )
                    nc.tensor.matmul(
                        xa_ps[th][:], AT_t[:, k], d_t[:],
                        start=(k == 0), stop=(k == KT - 1),
                    )
                    mm_count[0] += 1

            def xa_copy(th):
                nc.vector.tensor_copy(
                    xaT_t[:R, th * NF : (th + 1) * NF], xa_ps[th][:]
                )

            def close_group(ps, ot, th):
                ts = slice(th * NF, (th + 1) * NF)
                nc.tensor.matmul(
                    ps[:], BT_ts[ot][:], xaT_t[:, ts],
                    start=False, stop=True,
                )
                mm_count[0] += 1
                o_t = opool.tile([P, NF], f32, tag="o", name=f"o{ot}_{th}")
                nc.vector.tensor_copy(o_t[:], ps[:])
                nc.scalar.dma_start(out_d[ot, th], o_t[:])

            # ---- prologue: th0 x/mask pieces; staircase the five early
            # o-tiles' th0 k-runs so no matmul is issued before its W
            # tile can have arrived (W_i lands ~6us apart)
            backlog = {ot: 0 for ot in range(E5)}
            for kp in range(KP):
                dma_piece(0, kp)
                for ot in range(E5):
                    if ot <= kp + 1:
                        for k in range(backlog[ot], (kp + 1) * KPK):
                            mm(ps_early[ot], ot, 0, k, start=(k == 0))
                        backlog[ot] = (kp + 1) * KPK
                stage1_chunk(0, kp)

            # th1 x/mask stream (queued now, lands during phase B)
            for kp in range(KP):
                dma_piece(1, kp)

            xa_copy(0)
            # close the early o-tiles' th0 groups (frees their banks)
            for ot in range(E5):
                close_group(ps_early[ot], ot, 0)

            # ---- phase B: th0-only sweeps while th1 streams in;
            # inject LoRA stage-1 th1 chunks as their pieces arrive
            pump_w()
            pump_w()
            chunks = list(range(KP))
            nb = RB - E5
            for i, ot in enumerate(range(E5, RB)):
                pump_b()
                pump_w()
                ps = ps_mm.tile([P, NF], f32, tag="ps", name=f"ps{ot}_0")
                for k in range(KT):
                    mm(ps, ot, 0, k, start=(k == 0))
                # spread the 8 th1 chunks across the B o-tiles
                take = (KP * (i + 1)) // nb - (KP * i) // nb
                for _ in range(take):
                    stage1_chunk(1, chunks.pop(0))
                close_group(ps, ot, 0)
            assert not chunks
            xa_copy(1)

            # ---- phase C: both halves per o-tile
            for ot in range(RB, OT):
                pump_b()
                pump_w()
                for th in range(TH):
                    ps = ps_mm.tile([P, NF], f32, tag="ps", name=f"ps{ot}_{th}")
                    for k in range(KT):
                        mm(ps, ot, th, k, start=(k == 0))
                    close_group(ps, ot, th)

            # ---- phase D: th1 halves of the early o-tiles (W re-fetched)
            for ot in range(RB):
                pump_b()
                pump_w()
                ps = ps_mm.tile([P, NF], f32, tag="ps", name=f"ps{ot}_1b")
                for k in range(KT):
                    mm(ps, ot, 1, k, start=(k == 0))
                close_group(ps, ot, 1)

    for (ot, th), v in k_done.items():
        assert v == KT, (ot, th, v)
    assert mm_count[0] == OT * TH * (KT + 1) + TH * KT, mm_count[0]
    nc.finalize()
    return nc


def _get_program():
    if "nc" not in _PROGRAM_CACHE:
        _PROGRAM_CACHE["nc"] = _build_program()
    return _PROGRAM_CACHE["nc"]


def kernel(hidden_states, W_base, A, B, dropout_mask):
    from concourse.bass_utils import run_bass_kernel_spmd

    hs = np.ascontiguousarray(np.asarray(hidden_states, dtype=np.float32)).reshape(
        TOK, D_IN
    )
    mask = np.asarray(dropout_mask).reshape(TOK, D_IN)
    W = np.asarray(W_base, dtype=np.float32)
    A_ = np.asarray(A, dtype=np.float32)
    B_ = np.asarray(B, dtype=np.float32)

    #   WT[ot, p, k, o] = W[ot*P+o, k*P+p]
    WT = np.ascontiguousarray(
        W.reshape(OT, P, KT, P).transpose(0, 3, 2, 1)
    ).astype(np.float32)
    #   AT[p, k, r] = A[r, k*P+p] / (1-p)
    AT = np.ascontiguousarray(
        A_.T.reshape(KT, P, R).transpose(1, 0, 2) * np.float32(1.0 / (1.0 - DROP_P))
    ).astype(np.float32)
    #   BT[ot, r, o] = B[ot*P+o, r] * scaling, zero-padded to r=128
    BT = np.zeros((OT, P, P), dtype=np.float32)
    BT[:, :R, :] = B_.T.reshape(R, OT, P).transpose(1, 0, 2) * np.float32(SCALING)

    in_maps = []
    for c in range(NCORES):
        sl = slice(c * T, (c + 1) * T)
        #   xT[th, p, k, t] = x[c*T + th*NF + t, k*P+p]
        xc = hs[sl].T.reshape(KT, P, TH, NF).transpose(2, 1, 0, 3)
        mc = mask[sl].T.reshape(KT, P, TH, NF).transpose(2, 1, 0, 3)
        xT = np.ascontiguousarray(xc)
        mT = np.ascontiguousarray(mc).astype(np.uint8)
        in_maps.append({"xT": xT, "mT": mT, "WT": WT, "AT": AT, "BT": BT})

    nc = _get_program()
    res = run_bass_kernel_spmd(nc, in_maps, core_ids=list(range(NCORES)))
    _PROGRAM_CACHE["last_results"] = res

    # out_dev[ot, th, o, t] = out[th*NF+t, ot*P+o]  (per core)
    parts = []
    for c in range(NCORES):
        od = res.results[c]["out"]  # [OT, TH, P, NF]
        parts.append(od.transpose(1, 3, 0, 2).reshape(T, D_OUT))
    out = np.concatenate(parts, axis=0)
    return out.reshape(BATCH, SEQ, D_OUT).astype(np.float32)


# revision 18
# speedup vs baseline: 1.0297x; 1.0205x over previous
"""LoRA Linear (residual + low-rank path with dropout) on 8 Trainium2 cores.

Math (fp32 reference):
  residual = hidden_states @ W_base.T
  dropped  = hidden_states * dropout_mask / (1 - p)
  out      = residual + ((dropped @ A.T) @ B.T) * scaling

Sharding: data-parallel over the 8192 tokens (8 cores x 1024 tokens);
W_base / A / B replicated.  All matmuls run on the PE in float32r.

Layout strategy (v7 = v4 + dual-HWDGE-ring x/mask stream): the stationary operand is a W o-tile [128k x 128o]
and the moving operand is a 512-token slice of x [128k x 512t], PSUM
out = [128o x 512t].  At N=512 the per-matmul stream time (213 ns) has
slack over its f32r LDWEIGHTS (~190 ns with chase), and each PSUM bank
receives 33 back-to-back matmuls (k-loop + LoRA accumulate) before the
bank switches -- alternating banks per matmul breaks the PE's
LDWEIGHTS chase and serializes LDW->MM at ~272 ns/MM (measured).

The LoRA accumulate is zero-padded to K=128 (B rows 16..127 = 0, xa
rows 16..127 memset once) so every PE instruction is a homogeneous
[128x128] x [128x512] matmul -- a K=16 matmul at the group boundary
measured ~+400 ns per group (2x64 groups -> ~26 us).

Keeping x resident (128 KB/part) and streaming W in 16 KB/part o-tiles
keeps SBUF under the ~208 KB/part budget while W still streams exactly
once from HBM.  1/(1-p) is folded into A, `scaling` into B on the host.

Queues: x/mask stream on the SP HWDGE ring in 4-k-tile pieces; W0 on
the ACT HWDGE ring; the remaining W tiles and per-o-tile B tiles on the
Pool/GpSimd SWDGE ring; output DMAs on the ACT ring; DVE does the
mask-multiplies and PSUM drains.  The first three o-tiles' k-runs are
interleaved piece-by-piece with LoRA stage 1 so the PE has work while
x streams in.
"""

import numpy as np

P = 128
D_IN = 4096
D_OUT = 4096
BATCH, SEQ = 4, 2048
TOK = BATCH * SEQ  # 8192
NCORES = 8
T = TOK // NCORES  # 1024 tokens per core, all resident
KT = D_IN // P  # 32 k-tiles
OT = D_OUT // P  # 32 out-tiles of 128
TH = 2  # moving-dim halves (512 tokens each)
NF = T // TH  # 512 moving free dim
KP = 8  # x/mask DMA pieces (4 k-tiles each)
KPK = KT // KP
R = 16
DROP_P = 0.05
SCALING = 32.0 / 16.0
N_EARLY = 3  # o-tiles interleaved with the prologue
W_BUFS = 3
BT_BUFS = 4

_PROGRAM_CACHE = {}


def _build_program():
    from concourse import bacc
    import concourse.mybir as mybir
    import concourse.tile as tile

    f32 = mybir.dt.float32
    f32r = mybir.dt.float32r
    u8 = mybir.dt.uint8

    nc = bacc.Bacc("TRN2", target_bir_lowering=False)
    xT_d = nc.dram_tensor("xT", [P, KT, T], f32r, kind="ExternalInput")
    mT_d = nc.dram_tensor("mT", [P, KT, T], u8, kind="ExternalInput")
    WT_d = nc.dram_tensor("WT", [OT, P, KT, P], f32r, kind="ExternalInput")
    AT_d = nc.dram_tensor("AT", [P, KT, R], f32r, kind="ExternalInput")
    BT_d = nc.dram_tensor("BT", [OT, P, P], f32r, kind="ExternalInput")
    out_d = nc.dram_tensor("out", [OT, TH, P, NF], f32, kind="ExternalOutput")

    with tile.TileContext(nc) as tc:
        with (
            tc.tile_pool(name="xt", bufs=1) as xtpool,
            tc.tile_pool(name="at", bufs=1) as atpool,
            tc.tile_pool(name="bt", bufs=BT_BUFS) as btpool,
            tc.tile_pool(name="wt", bufs=W_BUFS) as wtpool,
            tc.tile_pool(name="m", bufs=3) as mpool,
            tc.tile_pool(name="d", bufs=2) as dpool,
            tc.tile_pool(name="xa", bufs=1) as xapool,
            tc.tile_pool(name="z", bufs=1) as zpool,
            tc.tile_pool(name="o", bufs=2) as opool,
            tc.tile_pool(name="ps_xa", bufs=2, space="PSUM") as ps_xa,
            tc.tile_pool(name="ps_mm", bufs=2 * N_EARLY, space="PSUM") as ps_mm,
        ):
            xT_t = xtpool.tile([P, KT, T], f32r, tag="xT")
            AT_t = atpool.tile([P, KT, R], f32r, tag="AT")
            nc.sync.dma_start(AT_t[:], AT_d[:])

            WT_ts = {}

            def load_w(ot, queue):
                WT_t = wtpool.tile([P, KT, P], f32r, tag="WT", name=f"WT{ot}")
                queue.dma_start(WT_t[:], WT_d[ot])
                WT_ts[ot] = WT_t

            BT_ts = {}

            def load_b(ot):
                BT_t = btpool.tile([P, P], f32r, tag="BT", name=f"BT{ot}")
                nc.gpsimd.dma_start(BT_t[:], BT_d[ot])
                BT_ts[ot] = BT_t

            # early W tiles: one per DMA ring so the SP ring carries only
            # the x/mask stream during startup
            for wi in range(W_BUFS):
                load_w(wi, nc.gpsimd)

            # xa, zero-padded to K=128 so the LoRA accumulate is a
            # homogeneous [128x128]x[128x512] matmul
            xaT_t = xapool.tile([P, T], f32r, tag="xaT")
            z_t = zpool.tile([P, NF], f32, tag="z")
            nc.vector.memset(z_t[:], 0.0)
            nc.vector.tensor_copy(xaT_t[:, :NF], z_t[:])
            nc.vector.tensor_copy(xaT_t[:, NF:], z_t[:])
            for ot in range(BT_BUFS):
                load_b(ot)

            xa_ps = [
                ps_xa.tile([R, NF], f32, tag="xa", name=f"xa_ps{h}")
                for h in range(TH)
            ]
            ps_early = {}
            for ot in range(N_EARLY):
                for th in range(TH):
                    ps_early[(ot, th)] = ps_mm.tile(
                        [P, NF], f32, tag="ps", name=f"ps{ot}_{th}"
                    )

            # ---- prologue: stream x/mask pieces; per piece run the first
            # three o-tiles' partial k-accumulation and LoRA stage 1
            # (xa += A.T @ (x*mask)) so the PE has work as x arrives.
            for kp in range(KP):
                ring = nc.sync if kp % 2 == 0 else nc.scalar
                ks = slice(kp * KPK, (kp + 1) * KPK)
                ring.dma_start(xT_t[:, ks], xT_d[:, ks])
                m_t = mpool.tile([P, KPK, T], u8, tag="m", name=f"m{kp}")
                ring.dma_start(m_t[:], mT_d[:, ks])
                for ot in range(N_EARLY):
                    for th in range(TH):
                        ts = slice(th * NF, (th + 1) * NF)
                        for k in range(kp * KPK, (kp + 1) * KPK):
                            nc.tensor.matmul(
                                ps_early[(ot, th)][:],
                                WT_ts[ot][:, k],
                                xT_t[:, k, ts],
                                start=(k == 0), stop=False,
                            )
                for th in range(TH):
                    ts = slice(th * NF, (th + 1) * NF)
                    for k in range(kp * KPK, (kp + 1) * KPK):
                        d_t = dpool.tile([P, NF], f32r, tag="d", name=f"d{k}_{th}")
                        nc.vector.tensor_tensor(
                            d_t[:], xT_t[:, k, ts].bitcast(f32),
                            m_t[:, k - kp * KPK, ts], mybir.AluOpType.mult,
                        )
                        nc.tensor.matmul(
                            xa_ps[th][:], AT_t[:, k], d_t[:],
                            start=(k == 0), stop=(k == KT - 1),
                        )

            for th in range(TH):
                nc.vector.tensor_copy(
                    xaT_t[:R, th * NF : (th + 1) * NF], xa_ps[th][:]
                )

            # ---- main loop: remaining o-tiles + LoRA accumulate + drain
            for ot in range(OT):
                if ot + BT_BUFS < OT:
                    load_b(ot + BT_BUFS)
                nxt = ot + W_BUFS
                if nxt < OT and nxt not in WT_ts:
                    load_w(nxt, nc.gpsimd)
                WT_t = WT_ts[ot]
                if ot < N_EARLY:
                    ps = [ps_early[(ot, th)] for th in range(TH)]
                else:
                    ps = [
                        ps_mm.tile([P, NF], f32, tag="ps", name=f"ps{ot}_{th}")
                        for th in range(TH)
                    ]
                for th in range(TH):
                    ts = slice(th * NF, (th + 1) * NF)
                    if ot >= N_EARLY:
                        for k in range(KT):
                            nc.tensor.matmul(
                                ps[th][:], WT_t[:, k], xT_t[:, k, ts],
                                start=(k == 0), stop=False,
                            )
                    nc.tensor.matmul(
                        ps[th][:], BT_ts[ot][:], xaT_t[:, ts],
                        start=False, stop=True,
                    )
                    o_t = opool.tile([P, NF], f32, tag="o", name=f"o{ot}_{th}")
                    nc.vector.tensor_copy(o_t[:], ps[th][:])
                    nc.scalar.dma_start(out_d[ot, th], o_t[:])

    nc.finalize()
    return nc


def _get_program():
    if "nc" not in _PROGRAM_CACHE:
        _PROGRAM_CACHE["nc"] = _build_program()
    return _PROGRAM_CACHE["nc"]


def kernel(hidden_states, W_base, A, B, dropout_mask):
    from concourse.bass_utils import run_bass_kernel_spmd

    hs = np.ascontiguousarray(np.asarray(hidden_states, dtype=np.float32)).reshape(
        TOK, D_IN
    )
    mask = np.asarray(dropout_mask).reshape(TOK, D_IN)
    W = np.asarray(W_base, dtype=np.float32)
    A_ = np.asarray(A, dtype=np.float32)
    B_ = np.asarray(B, dtype=np.float32)

    # Shared, pre-tiled weight layouts (fully contiguous per-partition DMA):
    #   WT[ot, p, k, o] = W[ot*P+o, k*P+p]
    WT = np.ascontiguousarray(
        W.reshape(OT, P, KT, P).transpose(0, 3, 2, 1)
    ).astype(np.float32)
    #   AT[p, k, r] = A[r, k*P+p] / (1-p)
    AT = np.ascontiguousarray(
        A_.T.reshape(KT, P, R).transpose(1, 0, 2) * np.float32(1.0 / (1.0 - DROP_P))
    ).astype(np.float32)
    #   BT[ot, r, o] = B[ot*P+o, r] * scaling, zero-padded to r=128
    BT = np.zeros((OT, P, P), dtype=np.float32)
    BT[:, :R, :] = B_.T.reshape(R, OT, P).transpose(1, 0, 2) * np.float32(SCALING)

    in_maps = []
    for c in range(NCORES):
        sl = slice(c * T, (c + 1) * T)
        #   xT[p, k, t] = x[c*T + t, k*P+p]
        xT = np.ascontiguousarray(
            hs[sl].T.reshape(KT, P, T).transpose(1, 0, 2)
        )
        mT = np.ascontiguousarray(
            mask[sl].T.reshape(KT, P, T).transpose(1, 0, 2)
        ).astype(np.uint8)
        in_maps.append({"xT": xT, "mT": mT, "WT": WT, "AT": AT, "BT": BT})

    nc = _get_program()
    res = run_bass_kernel_spmd(nc, in_maps, core_ids=list(range(NCORES)))
    _PROGRAM_CACHE["last_results"] = res

    # out_dev[ot, th, o, t] = out[th*NF+t, ot*P+o]  (per core)
    parts = []
    for c in range(NCORES):
        od = res.results[c]["out"]  # [OT, TH, P, NF]
        parts.append(od.transpose(1, 3, 0, 2).reshape(T, D_OUT))
    out = np.concatenate(parts, axis=0)
    return out.reshape(BATCH, SEQ, D_OUT).astype(np.float32)
